# revision 24
# baseline (speedup 1.0000x reference)
"""Two-layer LSTM (B=256, T=1000, H=128) on 8 TRN2 NeuronCores.

Strategy: data-parallel over batch (32 samples/core). All state kept
transposed [hid=128 partitions, batch=32 free]. Per step, gate
pre-activations for each layer are accumulated in one PSUM bank laid out
as [i|f|o|2g] (the g chunk pre-scaled by 2 so a single Sigmoid op covers
all four gates; tanh(g) = 2*sigmoid(2g)-1). Biases and the scalar input
x_t are folded into PSUM with small augmented matmuls (K=8 / K=4).
h2 history is DMA'd to HBM in chunks; the final 1-dim projection
(W_lin @ h2 + b_lin) is done on the host.
"""

import sys

sys.path.insert(0, "/opt/trn_rl_repo")

import numpy as np
from contextlib import ExitStack

import concourse.bass as bass
import concourse.bacc as bacc
import concourse.tile as tile
from concourse import mybir
from concourse.bass_utils import run_bass_kernel_spmd
from concourse.tile_rust import add_dep_helper as _add_dep_helper


def tile_rust_add_dep(later, earlier):
    """Order `later` after `earlier` (same engine: order-only, no sem)."""
    _add_dep_helper(later, earlier, sync=False, reason="keep L1 chain ahead")

H = 128
B = 256
T = 1000
NCORES = 8
BL = B // NCORES  # 32

F32 = mybir.dt.float32
F16 = mybir.dt.float16
AF = mybir.ActivationFunctionType
OP = mybir.AluOpType

# chunk order [i, f, o, g]; PyTorch row order is [i, f, g, o]
CH_SLICES = [slice(0, 128), slice(128, 256), slice(384, 512), slice(256, 384)]
CH_SCALES = [1.0, 1.0, 1.0, 2.0]

_nc_cache = {}
last_results = None  # BassKernelResults of the most recent run (for test.py)


def _chunk_cols(c):
    return slice(32 * c, 32 * c + 32)


def _build_nc(Tsteps, CH):
    """Build the SPMD Bass program for one core."""
    NBLK = (Tsteps + 2) // 3
    R = 2 * CH  # h2 ring length in steps

    nc = bacc.Bacc()
    xrep_h = nc.declare_dram_parameter("xrep", [128, NBLK * 128], F16, isOutput=False)
    consts_h = nc.declare_dram_parameter("consts", [128, 1920], F16, isOutput=False)
    h2out_h = nc.declare_dram_parameter("h2out", [128, Tsteps * 32], F16, isOutput=True)

    with tile.TileContext(nc) as tc, ExitStack() as ctx:
        cpool = ctx.enter_context(tc.tile_pool(name="const", bufs=1))
        xrep = cpool.tile([128, NBLK * 128], F16)
        consts = cpool.tile([128, 1920], F16)
        waug1 = consts[:, 0:128]
        whh1t = consts[:, 128:640]
        wih2t = consts[:, 640:1152]
        whh2t = consts[:, 1152:1664]
        b2aug = consts[0:4, 1664:1792]
        e4 = consts[0:4, 1792:1920]

        # consts split across the three DMA-capable engines' queues;
        # xrep: a tiny first slice gates step 0, the bulk streams in
        # behind the compute
        nc.sync.dma_start(out=consts[:, 0:640], in_=consts_h[:, 0:640])
        nc.scalar.dma_start(out=consts[:, 640:1280], in_=consts_h[:, 640:1280])
        nc.gpsimd.dma_start(out=consts[:, 1280:1920], in_=consts_h[:, 1280:1920])
        xcol = 0
        dma_engs = [nc.sync, nc.scalar, nc.gpsimd, nc.sync, nc.scalar, nc.gpsimd]
        for k, nblocks in enumerate((2, 16, 64, 84, 84, 84)):
            c0 = xcol * 128
            c1 = min(NBLK, xcol + nblocks) * 128
            if c0 >= c1:
                break
            dma_engs[k].dma_start(out=xrep[:, c0:c1], in_=xrep_h[:, c0:c1])
            xcol += nblocks

        # persistent state rings
        h1r = cpool.tile([128, 2 * 32], F16)
        c1r = cpool.tile([128, 2 * 32], F32)
        c2r = cpool.tile([128, 2 * 32], F32)
        h2r = cpool.tile([128, R * 32], F16)
        nc.vector.memset(h1r[:], 0.0)
        nc.vector.memset(c1r[:], 0.0)
        nc.vector.memset(c2r[:], 0.0)
        nc.vector.memset(h2r[:], 0.0)

        p1pool = ctx.enter_context(tc.tile_pool(name="p1", bufs=3, space="PSUM"))
        p2pool = ctx.enter_context(tc.tile_pool(name="p2", bufs=3, space="PSUM"))
        work = ctx.enter_context(tc.tile_pool(name="work", bufs=4))

        def aug_mm(p1, t):
            pb = 32 * (t % 3)
            blk = t // 3
            nc.tensor.matmul(
                p1[:],
                waug1[pb : pb + 8, :],
                xrep[pb : pb + 8, blk * 128 : blk * 128 + 128],
                start=True,
                stop=False,
            )

        # Software-pipelined: iteration t runs layer-1 of step t and
        # layer-2 of step t-1. Layer 1's loop (recs -> sigma -> cell ->
        # tanh -> h1') is the critical chain and gets queue priority on
        # PE/ACT/DVE; layer-2 elementwise rides GPSIMD where possible.
        p1_cur = p1pool.tile([128, 128], F32, tag="p1")
        aug_mm(p1_cur, 0)
        p2_cur = None  # psum2 tile for step t (bias+whh2 applied)
        p2_prev = None  # psum2 tile for step t-1
        i_u2_prev = None  # tanh2 instruction of the previous iteration
        for t in range(Tsteps + 1):
            s = (t % 2) * 32
            sp = ((t + 1) % 2) * 32
            h1prev = h1r[:, sp : sp + 32]

            # -- PE: layer-1 recurrent matmuls for step t (critical)
            if t < Tsteps:
                for c in range(4):
                    nc.tensor.matmul(
                        p1_cur[:, _chunk_cols(c)],
                        whh1t[:, 128 * c : 128 * c + 128],
                        h1prev,
                        start=False,
                        stop=(c == 3),
                    )
            # -- PE: finish layer-2 gates for step t-1
            if t >= 1:
                for c in range(4):
                    nc.tensor.matmul(
                        p2_prev[:, _chunk_cols(c)],
                        wih2t[:, 128 * c : 128 * c + 128],
                        h1prev,
                        start=False,
                        stop=False,
                    )
                h2pp = h2r[:, 32 * ((t - 2) % R) : 32 * ((t - 2) % R) + 32]
                for c in range(4):
                    nc.tensor.matmul(
                        p2_prev[:, _chunk_cols(c)],
                        whh2t[:, 128 * c : 128 * c + 128],
                        h2pp,
                        start=False,
                        stop=(c == 3),
                    )

            # -- layer-1 chain for step t
            # c' = sf*c + si*(2*s2g - 1) computed as (sf*c - si) + (2*s2g)*si
            # so only [STT m2a' -> TT c1' -> (tanh) -> TT h1'] sits on DVE.
            if t < Tsteps:
                sg1 = work.tile([128, 128], F32, tag="sg1")
                i_sg1 = nc.scalar.activation(sg1[:], p1_cur[:], AF.Sigmoid)
                if i_u2_prev is not None:
                    tile_rust_add_dep(i_sg1.ins, i_u2_prev.ins)
                # m2a = (2*s2g - 1) * si in ONE fused DVE op
                m2a = work.tile([128, 32], F32, tag="m2a")
                acc1 = work.tile([128, 1], F32, tag="acc1")
                nc.vector.affine_mul_reduce(
                    m2a[:], acc1[:], sg1[:, 96:128], sg1[:, 0:32], 2.0, -1.0
                )
                m1a = work.tile([128, 32], F32, tag="m1a")
                nc.gpsimd.tensor_tensor(
                    m1a[:], sg1[:, 32:64], c1r[:, sp : sp + 32], OP.mult
                )
                c1cur = c1r[:, s : s + 32]
                i_c1 = nc.vector.tensor_tensor(c1cur, m1a[:], m2a[:], OP.add)
                u1 = work.tile([128, 32], F32, tag="u1")
                i_u1 = nc.scalar.activation(u1[:], c1cur, AF.Tanh)
                h1cur = h1r[:, s : s + 32]
                i_h1 = nc.vector.tensor_tensor(h1cur, sg1[:, 64:96], u1[:], OP.mult)
            else:
                i_c1 = None
                i_sg1 = None
                i_u1 = None
                i_h1 = None

            # -- layer-2 chain for step t-1 (cell mostly on POOL)
            if t >= 1:
                sg2 = work.tile([128, 128], F32, tag="sg2")
                i_sg2 = nc.scalar.activation(sg2[:], p2_prev[:], AF.Sigmoid)
                if i_sg1 is not None:
                    tile_rust_add_dep(i_sg2.ins, i_sg1.ins)
                if i_u1 is not None:
                    tile_rust_add_dep(i_u1.ins, i_sg2.ins)
                m2b = work.tile([128, 32], F32, tag="m2b")
                acc2 = work.tile([128, 1], F32, tag="acc2")
                i_m2b = nc.vector.affine_mul_reduce(
                    m2b[:], acc2[:], sg2[:, 96:128], sg2[:, 0:32], 2.0, -1.0
                )
                if i_c1 is not None:
                    tile_rust_add_dep(i_m2b.ins, i_c1.ins)
                m1b = work.tile([128, 32], F32, tag="m1b")
                nc.gpsimd.tensor_tensor(
                    m1b[:], sg2[:, 32:64], c2r[:, s : s + 32], OP.mult
                )
                c2cur = c2r[:, sp : sp + 32]
                i_c2 = nc.vector.tensor_tensor(c2cur, m1b[:], m2b[:], OP.add)
                if i_h1 is not None:
                    tile_rust_add_dep(i_c2.ins, i_h1.ins)
                u2 = work.tile([128, 32], F32, tag="u2")
                i_u2 = nc.scalar.activation(u2[:], c2cur, AF.Tanh)
                if i_u1 is not None:
                    tile_rust_add_dep(i_u2.ins, i_u1.ins)
                i_u2_prev = i_u2
                h2cur = h2r[:, 32 * ((t - 1) % R) : 32 * ((t - 1) % R) + 32]
                nc.vector.tensor_tensor(h2cur, sg2[:, 64:96], u2[:], OP.mult)

            # -- PE prefills (off the chain)
            if t + 1 < Tsteps:
                p1_next = p1pool.tile([128, 128], F32, tag="p1")
                aug_mm(p1_next, t + 1)
            else:
                p1_next = None
            if t < Tsteps:
                p2_cur = p2pool.tile([128, 128], F32, tag="p2")
                nc.tensor.matmul(p2_cur[:], b2aug[:], e4[:], start=True, stop=False)
            else:
                p2_cur = None

            # -- DMA finished h2 chunks (steps up to t-1 are done)
            if t >= 1 and (t % CH == 0 or t == Tsteps):
                k = (t - 1) // CH
                t0 = k * CH
                t1 = t
                ra = 32 * (t0 % R)
                if t == Tsteps:
                    # final chunk: split across the three DMA queues so the
                    # kernel tail isn't one long packet train
                    n = t1 - t0
                    cuts = [0, n // 3, 2 * n // 3, n]
                    for eng, a, b in zip(
                        (nc.sync, nc.scalar, nc.gpsimd), cuts[:-1], cuts[1:]
                    ):
                        if b > a:
                            eng.dma_start(
                                out=h2out_h[:, 32 * (t0 + a) : 32 * (t0 + b)],
                                in_=h2r[:, ra + 32 * a : ra + 32 * b],
                            )
                else:
                    nc.sync.dma_start(
                        out=h2out_h[:, 32 * t0 : 32 * t1],
                        in_=h2r[:, ra : ra + 32 * (t1 - t0)],
                    )

            p1_cur = p1_next
            p2_prev = p2_cur

    nc.finalize()
    return nc


def _prep_shared(W_ih1, b_ih1, W_hh1, b_hh1, W_ih2, b_ih2, W_hh2, b_hh2):
    b1 = (b_ih1 + b_hh1).astype(np.float32)
    b2 = (b_ih2 + b_hh2).astype(np.float32)

    waug1 = np.zeros((128, 128), np.float32)
    whh1t = np.zeros((128, 512), np.float32)
    b2aug = np.zeros((4, 128), np.float32)
    wih2t = np.zeros((128, 512), np.float32)
    whh2t = np.zeros((128, 512), np.float32)
    e4 = np.zeros((4, 128), np.float32)
    for c in range(4):
        rows = CH_SLICES[c]
        sc = CH_SCALES[c]
        for base in (0, 32, 64):
            waug1[base + 2 * c] = sc * W_ih1[rows, 0]
            waug1[base + 2 * c + 1] = sc * b1[rows]
        whh1t[:, 128 * c : 128 * c + 128] = sc * W_hh1[rows, :].T
        b2aug[c] = sc * b2[rows]
        wih2t[:, 128 * c : 128 * c + 128] = sc * W_ih2[rows, :].T
        whh2t[:, 128 * c : 128 * c + 128] = sc * W_hh2[rows, :].T
        e4[c, 32 * c : 32 * c + 32] = 1.0
    consts = np.zeros((128, 1920), np.float32)
    consts[:, 0:128] = waug1
    consts[:, 128:640] = whh1t
    consts[:, 640:1152] = wih2t
    consts[:, 1152:1664] = whh2t
    consts[0:4, 1664:1792] = b2aug
    consts[0:4, 1792:1920] = e4
    return consts.astype(np.float16)


def _prep_xrep(x_shard, Tsteps):
    """x_shard [BL, T] -> xrep [128, NBLK*128] fp16.

    Step t lives at partition base 32*(t%3), col block t//3. Within the
    [8, 128] block: row 2c carries x_t replicated in col-chunk c, row
    2c+1 carries ones in col-chunk c (zero elsewhere).
    """
    NBLK = (Tsteps + 2) // 3
    xrep = np.zeros((128, NBLK * 128), np.float16)
    bcols = np.arange(32)
    for k in range(3):
        tidx = np.arange(NBLK) * 3 + k
        valid = tidx < Tsteps
        xr = np.zeros((NBLK, 32), np.float32)
        xr[valid] = x_shard[:, tidx[valid]].T
        ones = np.zeros((NBLK, 32), np.float32)
        ones[valid] = 1.0
        for c in range(4):
            cols = (np.arange(NBLK)[:, None] * 128 + 32 * c + bcols[None, :]).ravel()
            xrep[32 * k + 2 * c, cols] = xr.ravel().astype(np.float16)
            xrep[32 * k + 2 * c + 1, cols] = ones.ravel().astype(np.float16)
    return xrep


def kernel(
    input,
    W_ih1,
    b_ih1,
    W_hh1,
    b_hh1,
    W_ih2,
    b_ih2,
    W_hh2,
    b_hh2,
    W_lin,
    b_lin,
    _Tsteps=None,
    _trace=False,
):
    global last_results
    Tsteps = T if _Tsteps is None else _Tsteps
    CH = 25 if Tsteps >= 50 else max(1, Tsteps // 2)

    input = np.asarray(input, np.float32)
    key = (Tsteps, CH)
    if key not in _nc_cache:
        _nc_cache[key] = _build_nc(Tsteps, CH)
    nc = _nc_cache[key]

    consts = _prep_shared(
        np.asarray(W_ih1, np.float32),
        np.asarray(b_ih1, np.float32),
        np.asarray(W_hh1, np.float32),
        np.asarray(b_hh1, np.float32),
        np.asarray(W_ih2, np.float32),
        np.asarray(b_ih2, np.float32),
        np.asarray(W_hh2, np.float32),
        np.asarray(b_hh2, np.float32),
    )
    in_maps = []
    for ci in range(NCORES):
        x_shard = input[ci * BL : (ci + 1) * BL, :Tsteps]
        in_maps.append({"xrep": _prep_xrep(x_shard, Tsteps), "consts": consts})

    res = run_bass_kernel_spmd(nc, in_maps, core_ids=list(range(NCORES)), trace=_trace)
    last_results = res

    wl = np.asarray(W_lin, np.float32)[0]  # [128]
    bl = float(np.asarray(b_lin, np.float32)[0])
    out = np.empty((B, Tsteps), np.float32)
    for ci in range(NCORES):
        h2 = np.asarray(res.results[ci]["h2out"], np.float32).reshape(128, Tsteps, 32)
        out[ci * BL : (ci + 1) * BL] = np.einsum("j,jtb->bt", wl, h2) + bl
    return out
